# revision 35
# baseline (speedup 1.0000x reference)
"""Trainium2 Bass kernel for nn_FCLModule_74131135529089 (moe_routing).

Module structure (B=262144 rows, input dim 1):
    circle/rect expert towers 1->32->64->256 (relu, zero biases)
    per-row select by shape_type, stage2 256->256 relu + residual,
    stage3 256->512 relu, 512->512, LayerNorm(512).

All bias vectors in this module are zero and every stage before the
LayerNorm is positively homogeneous in x: for each row,
    h2(x) = |x| * H[k],   k = 2*shape_type + (x < 0),
where H[k] in R^512 is the full pre-LayerNorm output of the network at
x = +-1 for each expert.  The LayerNorm collapses to
    out = C[k] * t + ln_b,  C[k] = (H[k]-mean(H[k]))*ln_g,
    t = |x| / sqrt(x^2 * var(H[k]) + eps).

Device kernel: per 128-row chunk build a [5, 128] fp16 stationary
(4 one-hot*t rows + ones row) and multiply with the constant [5, 512]
fp16 matrix (C rows + ln_b) on the tensor engine; accumulate fp32 in
PSUM, drain to SBUF as fp16 (ACT/DVE alternating), DMA fp16 to HBM and
upcast to fp32 on the host.  fp16 output keeps rel err ~1e-3, well
under the 2e-2 gate, and halves HBM write traffic (the roofline).

The stationaries are produced without tensor-engine transposes: inputs
are host-permuted so that the per-row coefficient planes, written by
the vector engine as [128, 4, 32]-blocked tiles, become per-chunk
stationaries under the DVE 32x32 StreamTranspose (one per 4 chunks).

Sharding: pure data parallel over the batch dim, 8 cores x 32768 rows.
If structural assumptions are violated (nonzero biases / shape_type
outside {0,1}) we fall back to dense numpy evaluation.
"""

import numpy as np

B = 262144
TD = 512
N_CORES = 8
RPC = B // N_CORES          # rows per core = 32768
P = 128                     # SBUF partitions
CPB = RPC // P              # chunks per core = 256 (row r = p*CPB + c)
TB = 4                      # chunks per transpose batch
NB = CPB // TB              # transpose batches = 64
KD = 32                     # stream-transpose block / stationary slot
KU = 5                      # used stationary rows (4 masks*t + ones)
G = 32                      # chunks per output DMA group
EPS = 1e-5

_CACHE: dict = {}


def _towers_collapse(inputs):
    """Host-side constant folding (float64): returns the replicated fp16
    constant matrix [128, TD] (rows 32j+k: k<4 -> C[k], k=4 -> ln_b) and
    sig2 [4] f64, for k = 2*shape_type + (x<0) in order
    (c,+),(c,-),(r,+),(r,-)."""
    W = {k: np.asarray(v, dtype=np.float64) for k, v in inputs.items()}
    H = []
    for e in ("c", "r"):
        for sign in (1.0, -1.0):
            v = np.array([[sign]])
            for li in ("1", "2", "3"):
                v = np.maximum(v @ W[e + "w" + li] + W[e + "b" + li], 0.0)
            x2 = np.maximum(v @ W["s2w"] + W["s2b"], 0.0) + v
            h = np.maximum(x2 @ W["w3a"] + W["b3a"], 0.0)
            H.append((h @ W["w3b"] + W["b3b"])[0])
    H = np.stack(H)                                   # [4, TD]
    mu = H.mean(axis=1, keepdims=True)
    sig2 = H.var(axis=1)                              # [4]
    C = (H - mu) * W["ln_g"][None, :]                 # [4, TD]
    # one-hot basis: plane k = t*mask_k keeps the fp16 rounding error
    # relative to the single active C row (affine differences would not)
    blk = np.zeros((KD, TD), np.float16)
    blk[0:4] = C.astype(np.float16)
    blk[4] = W["ln_b"].astype(np.float16)
    cmat = np.tile(blk, (TB, 1))                      # [128, TD] f16
    return np.ascontiguousarray(cmat), sig2


def _assumptions_hold(inputs):
    for name in ("cb1", "cb2", "cb3", "rb1", "rb2", "rb3", "s2b", "b3a", "b3b"):
        if np.any(np.asarray(inputs[name]) != 0):
            return False
    st = np.asarray(inputs["shape_type"])
    if not np.isin(st, (0, 1)).all():
        return False
    x = np.asarray(inputs["x"])
    return bool(np.isfinite(x).all()) and x.shape == (B, 1) and st.shape == (B, 1)


def _fallback_numpy(inputs):
    f = {k: np.asarray(v, dtype=np.float32) for k, v in inputs.items()}

    def tower(h, w1, b1, w2, b2, w3, b3):
        h = np.maximum(h @ w1 + b1, 0)
        h = np.maximum(h @ w2 + b2, 0)
        return np.maximum(h @ w3 + b3, 0)

    x = f["x"]
    circle = tower(x, f["cw1"], f["cb1"], f["cw2"], f["cb2"], f["cw3"], f["cb3"])
    rect = tower(x, f["rw1"], f["rb1"], f["rw2"], f["rb2"], f["rw3"], f["rb3"])
    mask = np.asarray(inputs["shape_type"]) < 0.5
    x1 = np.where(mask, circle, rect)
    x2 = np.maximum(x1 @ f["s2w"] + f["s2b"], 0) + x1
    h = np.maximum(x2 @ f["w3a"] + f["b3a"], 0)
    h = h @ f["w3b"] + f["b3b"]
    mu = h.mean(axis=-1, keepdims=True)
    var = h.var(axis=-1, keepdims=True)
    return ((h - mu) / np.sqrt(var + EPS) * f["ln_g"] + f["ln_b"]).astype(np.float32)


def _build_nc(sig2):
    import concourse.bacc as bacc
    import concourse.bass as bass
    import concourse.mybir as mybir
    import concourse.tile as tile

    f32 = mybir.dt.float32
    f16 = mybir.dt.float16
    a = float(sig2[0])
    b = float(sig2[1] - sig2[0])
    c = float(sig2[2] - sig2[0])
    d = float(sig2[3] - sig2[2] - sig2[1] + sig2[0])
    mul = mybir.AluOpType.mult
    add = mybir.AluOpType.add
    sub = mybir.AluOpType.subtract

    nc = bacc.Bacc("TRN2", target_bir_lowering=False, debug=False,
                   num_devices=N_CORES)
    # host-permuted inputs: element [q, t, rb] = row (32*rb + q%32)*CPB
    # + 4*t + q//32  (q = 32j + a selects chunk-within-batch j, row-mod a)
    x_d = nc.dram_tensor("x", [P, NB, TB], f32, kind="ExternalInput").ap()
    s_d = nc.dram_tensor("st", [P, NB, TB], f32, kind="ExternalInput").ap()
    c_d = nc.dram_tensor("cmat", [P, TD], f16, kind="ExternalInput").ap()
    y_d = nc.dram_tensor("y", [P, CPB, TD], f16, kind="ExternalOutput").ap()

    # output DMA groups
    groups = [(0, 4), (4, 4), (8, 8), (16, 16), (32, 32)]
    g0 = 64
    while g0 < CPB:
        groups.append((g0, G))
        g0 += G
    # prep slices (chunk ranges).  Small head slices run fully on DVE for
    # latency; big tail slices run on GpSimd (idle engine) so DVE keeps
    # only transposes + PSUM drains, immune to the shared-SBUF-port
    # contention GpSimd inflicts on DVE SBUF-read ops.  GpSimd slices are
    # split: phase A (chain up to ve) emitted EARLY, phase B (sqrt +
    # planes) emitted LATE so the in-order ACT queue never head-of-line
    # blocks drains behind a sqrt whose input chain is still running.
    slices = [(0, 4), (4, 4), (8, 8), (16, 16),
              (32, 64), (96, 64), (160, 96)]
    N_DVE_SLICES = 4
    # emission schedule: group index -> list of (slice_idx, phase) where
    # phase is "ab" (both), "a", or "b"
    prep_at = {0: [(2, "ab")], 1: [(3, "ab"), (4, "b")],
               2: [(5, "a")], 3: [(5, "b")], 4: [(6, "a")],
               6: [(6, "b")]}

    with tile.TileContext(nc) as tc:
        with (
            tc.tile_pool(name="const", bufs=1) as const,
            tc.tile_pool(name="pre", bufs=1) as pre,
            tc.tile_pool(name="lhs", bufs=4) as lhsp,
            tc.tile_pool(name="outs", bufs=3) as outp,
            tc.tile_pool(name="ps", bufs=2, space="PSUM") as psp,
        ):
            xr = pre.tile([P, NB, TB], f32)
            sf = pre.tile([P, NB, TB], f32)
            neg = pre.tile([P, NB, TB], f32)
            sn = pre.tile([P, NB, TB], f32)
            u1 = pre.tile([P, NB, TB], f32)
            u2 = pre.tile([P, NB, TB], f32)
            sg = pre.tile([P, NB, TB], f32)
            x2 = pre.tile([P, NB, TB], f32)
            ve = pre.tile([P, NB, TB], f32)
            rc = pre.tile([P, NB, TB], f32)
            rq = pre.tile([P, NB, TB], f32)
            tt = pre.tile([P, NB, TB], f32)
            v1 = pre.tile([P, NB, TB], f32)
            v2 = pre.tile([P, NB, TB], f32)
            v3 = pre.tile([P, NB, TB], f32)
            q0 = pre.tile([P, NB, TB], f32)
            m4 = pre.tile([P, NB, TB, KD], f16)
            wv = pre.tile([P, 8], f32)
            wa = pre.tile([P, 8], f32)

            # input DMAs: first prep slice's columns first
            nc.sync.dma_start(xr[:, 0:1, :], x_d[:, 0:1, :])
            nc.scalar.dma_start(sf[:, 0:1, :], s_d[:, 0:1, :])
            nc.sync.dma_start(xr[:, 1:NB, :], x_d[:, 1:NB, :])
            nc.scalar.dma_start(sf[:, 1:NB, :], s_d[:, 1:NB, :])
            cm = const.tile([P, TD], f16)
            nc.scalar.dma_start(cm[:], c_d[:])

            # engine warmups during input-DMA latency: wake DVE/GpSimd, and
            # load the ACT Copy+Rsqrt tables before first real use
            nc.vector.memset(wv[:], 1.0)
            nc.vector.tensor_scalar(wv[:], wv[:], 1.0, None, mul)
            nc.vector.tensor_tensor(wv[:], wv[:], wv[:], mul)
            nc.gpsimd.tensor_scalar(wa[:], wv[:], 1.0, None, mul)
            nc.scalar.copy(wa[:], wv[:])
            nc.scalar.activation(wa[:], wv[:],
                                 mybir.ActivationFunctionType.Sqrt)

            # m4 pad planes (k in [KU,KD)) are left uninitialized: only the
            # NaN-safe StreamTranspose reads them, and the K=5 matmuls
            # never touch lh rows 5-31.

            drain_ctr = [0]
            DRAIN_PAT = (0, 1, 0, 0, 1)   # 0 = ACT, 1 = DVE
            N_RAMP_ACT = 2                # first drains forced to ACT

            def emit_prep_a(si):
                c0, gsz = slices[si]
                cs = slice(c0 // TB, (c0 + gsz) // TB)
                e = nc.vector if si < N_DVE_SLICES else nc.gpsimd
                # neg = x<0 ; sig2 = (a + b*neg) + st*(c + d*neg)
                e.tensor_scalar(neg[:, cs, :], xr[:, cs, :], 0.0,
                                None, mybir.AluOpType.is_lt)
                e.tensor_scalar(u1[:, cs, :], neg[:, cs, :], b, a,
                                mul, add)
                e.tensor_scalar(u2[:, cs, :], neg[:, cs, :], d, c,
                                mul, add)
                e.tensor_tensor(sn[:, cs, :], sf[:, cs, :],
                                u2[:, cs, :], mul)
                e.tensor_tensor(sg[:, cs, :], u1[:, cs, :],
                                sn[:, cs, :], add)
                e.tensor_tensor(x2[:, cs, :], xr[:, cs, :],
                                xr[:, cs, :], mul)
                e.tensor_tensor(ve[:, cs, :], x2[:, cs, :],
                                sg[:, cs, :], mul)
                e.tensor_scalar(ve[:, cs, :], ve[:, cs, :], EPS,
                                None, add)

            def emit_prep_b(si):
                c0, gsz = slices[si]
                cs = slice(c0 // TB, (c0 + gsz) // TB)
                e = nc.vector if si < N_DVE_SLICES else nc.gpsimd
                # t = sqrt(x^2 / (x^2*sig2 + eps))
                nc.vector.reciprocal(rc[:, cs, :], ve[:, cs, :])
                e.tensor_tensor(rq[:, cs, :], x2[:, cs, :], rc[:, cs, :],
                                mul)
                nc.scalar.activation(tt[:, cs, :], rq[:, cs, :],
                                     mybir.ActivationFunctionType.Sqrt)
                # one-hot planes: p0 = t(1-s)(1-n), p1 = t(1-s)n,
                # p2 = ts(1-n), p3 = tsn, p4 = 1
                e.tensor_tensor(v1[:, cs, :], tt[:, cs, :],
                                neg[:, cs, :], mul)
                e.tensor_tensor(v2[:, cs, :], tt[:, cs, :],
                                sf[:, cs, :], mul)
                e.tensor_tensor(v3[:, cs, :], v2[:, cs, :],
                                neg[:, cs, :], mul)
                e.tensor_copy(m4[:, cs, :, 3], v3[:, cs, :])
                e.tensor_tensor(m4[:, cs, :, 1], v1[:, cs, :],
                                v3[:, cs, :], sub)
                e.tensor_tensor(m4[:, cs, :, 2], v2[:, cs, :],
                                v3[:, cs, :], sub)
                e.tensor_tensor(q0[:, cs, :], tt[:, cs, :],
                                v1[:, cs, :], sub)
                e.tensor_tensor(q0[:, cs, :], q0[:, cs, :],
                                v2[:, cs, :], sub)
                e.tensor_tensor(m4[:, cs, :, 0], q0[:, cs, :],
                                v3[:, cs, :], add)
                e.memset(m4[:, cs, :, 4], 1.0)

            def emit_prep(si, phase="ab"):
                if "a" in phase:
                    emit_prep_a(si)
                if "b" in phase:
                    emit_prep_b(si)

            def emit_group(h):
                c0, gsz = groups[h]
                outt = outp.tile([P, G, TD], f16, tag="outt")
                pend = {}
                for t0 in range(0, gsz, TB):
                    t = (c0 + t0) // TB
                    if t not in pend:
                        # one StreamTranspose covers 2 batches (blocks are
                        # independent) -- halves DVE op + semaphore count
                        lh = lhsp.tile([P, 2 * P], f16, tag="lh")
                        if gsz - t0 >= 2 * TB:
                            nc.vector.transpose(lh[:], m4[:, t:t + 2, :, :])
                            pend[t] = (lh, 0)
                            pend[t + 1] = (lh, P)
                        else:
                            nc.vector.transpose(lh[:, 0:P], m4[:, t, :, :])
                            pend[t] = (lh, 0)
                    lh, off = pend.pop(t)
                    pp = psp.tile([P, TB, TD], f32, tag="pp")
                    for j in range(TB):
                        nc.tensor.matmul(
                            pp[:, j, :],
                            lh[KD * j:KD * j + KU, off:off + P],
                            cm[KD * j:KD * j + KU, :],
                            start=True, stop=True,
                            tile_position=(KD * j, 0))
                    dst = outt[:, t0:t0 + TB, :]
                    ctr = drain_ctr[0]
                    use_dve = (ctr >= N_RAMP_ACT
                               and DRAIN_PAT[ctr % len(DRAIN_PAT)])
                    if use_dve:
                        nc.vector.tensor_copy(dst, pp[:])
                    else:
                        nc.scalar.copy(dst, pp[:])
                    drain_ctr[0] += 1
                    if t0 == 0:
                        for si, phase in prep_at.get(h, ()):
                            emit_prep(si, phase)
                nc.sync.dma_start(y_d[:, c0:c0 + gsz, :], outt[:, 0:gsz, :])

            emit_prep(0)
            emit_prep(1)
            emit_prep(4, "a")
            for h in range(len(groups)):
                emit_group(h)
    nc.compile()
    return nc


def _make_in_maps(inputs, cmat):
    x = np.ascontiguousarray(np.asarray(inputs["x"], dtype=np.float32)).reshape(B)
    st = np.asarray(inputs["shape_type"]).astype(np.float32).reshape(B)
    in_maps = []
    for i in range(N_CORES):
        sl = slice(i * RPC, (i + 1) * RPC)
        # [q = 32j+a, t, rb] <- row (32rb + a)*CPB + 4t + j
        xq = x[sl].reshape(TB, 32, NB, TB).transpose(3, 1, 2, 0).reshape(
            P, NB, TB)
        sq = st[sl].reshape(TB, 32, NB, TB).transpose(3, 1, 2, 0).reshape(
            P, NB, TB)
        in_maps.append({
            "x": np.ascontiguousarray(xq),
            "st": np.ascontiguousarray(sq),
            "cmat": cmat,
        })
    return in_maps


def _get_nc(sig2):
    key = tuple(np.round(sig2, 12))
    if key not in _CACHE:
        _CACHE[key] = _build_nc(sig2)
    return _CACHE[key]


def _get_runner(nc):
    """Cached jit-compiled SPMD executor for `nc` (same mechanics as
    concourse.bass2jax.run_bass_via_pjrt, memoized so repeated kernel()
    calls skip jax re-tracing)."""
    if hasattr(nc, "_cached_runner"):
        return nc._cached_runner
    import jax
    from jax.experimental.shard_map import shard_map
    from jax.sharding import Mesh, PartitionSpec

    import concourse.mybir as mybir
    from concourse import bass2jax

    bass2jax.install_neuronx_cc_hook()

    part_name = (nc.partition_id_tensor.name
                 if nc.partition_id_tensor else None)
    in_names, out_names, out_avals = [], [], []
    for alloc in nc.m.functions[0].allocations:
        if not isinstance(alloc, mybir.MemoryLocationSet):
            continue
        name = alloc.memorylocations[0].name
        if alloc.kind == "ExternalInput":
            if name != part_name:
                in_names.append(name)
        elif alloc.kind == "ExternalOutput":
            out_names.append(name)
            out_avals.append(jax.core.ShapedArray(
                tuple(alloc.tensor_shape), mybir.dt.np(alloc.dtype)))
    n_params = len(in_names)
    all_names = in_names + out_names
    if part_name is not None:
        all_names = all_names + [part_name]
    donate = tuple(range(n_params, n_params + len(out_names)))

    def _body(*args):
        operands = list(args)
        if part_name is not None:
            operands.append(bass2jax.partition_id_tensor())
        return tuple(bass2jax._bass_exec_p.bind(
            *operands,
            out_avals=tuple(out_avals),
            in_names=tuple(all_names),
            out_names=tuple(out_names),
            lowering_input_output_aliases=(),
            sim_require_finite=True,
            sim_require_nnan=True,
            nc=nc,
        ))

    devices = jax.devices()[:N_CORES]
    mesh = Mesh(np.asarray(devices), ("core",))
    sharded = jax.jit(
        shard_map(_body, mesh=mesh,
                  in_specs=(PartitionSpec("core"),) * (n_params + len(out_names)),
                  out_specs=(PartitionSpec("core"),) * len(out_names),
                  check_rep=False),
        donate_argnums=donate, keep_unused=True)
    runner = (sharded, in_names, out_names, out_avals)
    nc._cached_runner = runner
    return runner


def _run_spmd(nc, in_maps):
    sharded, in_names, out_names, out_avals = _get_runner(nc)
    concat_in = [
        np.concatenate([np.asarray(m[name])[None] for m in in_maps], axis=0)
        .reshape(N_CORES * in_maps[0][name].shape[0],
                 *in_maps[0][name].shape[1:])
        for name in in_names
    ]
    concat_zeros = [
        np.zeros((N_CORES * a.shape[0], *a.shape[1:]), a.dtype)
        for a in out_avals
    ]
    out_arrs = sharded(*concat_in, *concat_zeros)
    return {
        name: np.asarray(out_arrs[i]).reshape(
            N_CORES, *out_avals[i].shape)
        for i, name in enumerate(out_names)
    }


def kernel(**inputs) -> np.ndarray:
    if not _assumptions_hold(inputs):
        return _fallback_numpy(inputs)

    cmat, sig2 = _towers_collapse(inputs)
    nc = _get_nc(sig2)
    in_maps = _make_in_maps(inputs, cmat)
    y = _run_spmd(nc, in_maps)["y"]            # [N_CORES, P, CPB, TD] f16
    return np.ascontiguousarray(y.reshape(B, TD)).astype(np.float32)


# revision 36
# speedup vs baseline: 1.0238x; 1.0238x over previous
"""Trainium2 Bass kernel for nn_FCLModule_74131135529089 (moe_routing).

Module structure (B=262144 rows, input dim 1):
    circle/rect expert towers 1->32->64->256 (relu, zero biases)
    per-row select by shape_type, stage2 256->256 relu + residual,
    stage3 256->512 relu, 512->512, LayerNorm(512).

All bias vectors in this module are zero and every stage before the
LayerNorm is positively homogeneous in x: for each row,
    h2(x) = |x| * H[k],   k = 2*shape_type + (x < 0),
where H[k] in R^512 is the full pre-LayerNorm output of the network at
x = +-1 for each expert.  The LayerNorm collapses to
    out = C[k] * t + ln_b,  C[k] = (H[k]-mean(H[k]))*ln_g,
    t = |x| / sqrt(x^2 * var(H[k]) + eps).

Device kernel: per 128-row chunk build a [5, 128] fp16 stationary
(4 one-hot*t rows + ones row) and multiply with the constant [5, 512]
fp16 matrix (C rows + ln_b) on the tensor engine; accumulate fp32 in
PSUM, drain to SBUF as fp16 (ACT/DVE alternating), DMA fp16 to HBM and
upcast to fp32 on the host.  fp16 output keeps rel err ~1e-3, well
under the 2e-2 gate, and halves HBM write traffic (the roofline).

The stationaries are produced without tensor-engine transposes: inputs
are host-permuted so that the per-row coefficient planes, written by
the vector engine as [128, 4, 32]-blocked tiles, become per-chunk
stationaries under the DVE 32x32 StreamTranspose (one per 4 chunks).

Sharding: pure data parallel over the batch dim, 8 cores x 32768 rows.
If structural assumptions are violated (nonzero biases / shape_type
outside {0,1}) we fall back to dense numpy evaluation.
"""

import numpy as np

B = 262144
TD = 512
N_CORES = 8
RPC = B // N_CORES          # rows per core = 32768
P = 128                     # SBUF partitions
CPB = RPC // P              # chunks per core = 256 (row r = p*CPB + c)
TB = 4                      # chunks per transpose batch
NB = CPB // TB              # transpose batches = 64
KD = 32                     # stream-transpose block / stationary slot
KU = 5                      # used stationary rows (4 masks*t + ones)
G = 32                      # chunks per output DMA group
EPS = 1e-5

_CACHE: dict = {}


def _towers_collapse(inputs):
    """Host-side constant folding (float64): returns the replicated fp16
    constant matrix [128, TD] (rows 32j+k: k<4 -> C[k], k=4 -> ln_b) and
    sig2 [4] f64, for k = 2*shape_type + (x<0) in order
    (c,+),(c,-),(r,+),(r,-)."""
    W = {k: np.asarray(v, dtype=np.float64) for k, v in inputs.items()}
    H = []
    for e in ("c", "r"):
        for sign in (1.0, -1.0):
            v = np.array([[sign]])
            for li in ("1", "2", "3"):
                v = np.maximum(v @ W[e + "w" + li] + W[e + "b" + li], 0.0)
            x2 = np.maximum(v @ W["s2w"] + W["s2b"], 0.0) + v
            h = np.maximum(x2 @ W["w3a"] + W["b3a"], 0.0)
            H.append((h @ W["w3b"] + W["b3b"])[0])
    H = np.stack(H)                                   # [4, TD]
    mu = H.mean(axis=1, keepdims=True)
    sig2 = H.var(axis=1)                              # [4]
    C = (H - mu) * W["ln_g"][None, :]                 # [4, TD]
    # one-hot basis: plane k = t*mask_k keeps the fp16 rounding error
    # relative to the single active C row (affine differences would not)
    blk = np.zeros((KD, TD), np.float16)
    blk[0:4] = C.astype(np.float16)
    blk[4] = W["ln_b"].astype(np.float16)
    cmat = np.tile(blk, (TB, 1))                      # [128, TD] f16
    return np.ascontiguousarray(cmat), sig2


def _assumptions_hold(inputs):
    for name in ("cb1", "cb2", "cb3", "rb1", "rb2", "rb3", "s2b", "b3a", "b3b"):
        if np.any(np.asarray(inputs[name]) != 0):
            return False
    st = np.asarray(inputs["shape_type"])
    if not np.isin(st, (0, 1)).all():
        return False
    x = np.asarray(inputs["x"])
    return bool(np.isfinite(x).all()) and x.shape == (B, 1) and st.shape == (B, 1)


def _fallback_numpy(inputs):
    f = {k: np.asarray(v, dtype=np.float32) for k, v in inputs.items()}

    def tower(h, w1, b1, w2, b2, w3, b3):
        h = np.maximum(h @ w1 + b1, 0)
        h = np.maximum(h @ w2 + b2, 0)
        return np.maximum(h @ w3 + b3, 0)

    x = f["x"]
    circle = tower(x, f["cw1"], f["cb1"], f["cw2"], f["cb2"], f["cw3"], f["cb3"])
    rect = tower(x, f["rw1"], f["rb1"], f["rw2"], f["rb2"], f["rw3"], f["rb3"])
    mask = np.asarray(inputs["shape_type"]) < 0.5
    x1 = np.where(mask, circle, rect)
    x2 = np.maximum(x1 @ f["s2w"] + f["s2b"], 0) + x1
    h = np.maximum(x2 @ f["w3a"] + f["b3a"], 0)
    h = h @ f["w3b"] + f["b3b"]
    mu = h.mean(axis=-1, keepdims=True)
    var = h.var(axis=-1, keepdims=True)
    return ((h - mu) / np.sqrt(var + EPS) * f["ln_g"] + f["ln_b"]).astype(np.float32)


def _build_nc(sig2):
    import concourse.bacc as bacc
    import concourse.bass as bass
    import concourse.mybir as mybir
    import concourse.tile as tile

    f32 = mybir.dt.float32
    f16 = mybir.dt.float16
    a = float(sig2[0])
    b = float(sig2[1] - sig2[0])
    c = float(sig2[2] - sig2[0])
    d = float(sig2[3] - sig2[2] - sig2[1] + sig2[0])
    mul = mybir.AluOpType.mult
    add = mybir.AluOpType.add
    sub = mybir.AluOpType.subtract

    nc = bacc.Bacc("TRN2", target_bir_lowering=False, debug=False,
                   num_devices=N_CORES)
    # host-permuted inputs: element [q, t, rb] = row (32*rb + q%32)*CPB
    # + 4*t + q//32  (q = 32j + a selects chunk-within-batch j, row-mod a)
    x_d = nc.dram_tensor("x", [P, NB, TB], f32, kind="ExternalInput").ap()
    s_d = nc.dram_tensor("st", [P, NB, TB], f32, kind="ExternalInput").ap()
    c_d = nc.dram_tensor("cmat", [P, TD], f16, kind="ExternalInput").ap()
    y_d = nc.dram_tensor("y", [P, CPB, TD], f16, kind="ExternalOutput").ap()

    # output DMA groups
    groups = [(0, 4), (4, 4), (8, 8), (16, 16), (32, 32)]
    g0 = 64
    while g0 < CPB:
        groups.append((g0, G))
        g0 += G
    # prep slices (chunk ranges).  Small head slices run fully on DVE for
    # latency; big tail slices run on GpSimd (idle engine) so DVE keeps
    # only transposes + PSUM drains, immune to the shared-SBUF-port
    # contention GpSimd inflicts on DVE SBUF-read ops.  GpSimd slices are
    # split: phase A (chain up to ve) emitted EARLY, phase B (sqrt +
    # planes) emitted LATE so the in-order ACT queue never head-of-line
    # blocks drains behind a sqrt whose input chain is still running.
    slices = [(0, 4), (4, 4), (8, 8), (16, 16),
              (32, 64), (96, 64), (160, 96)]
    N_DVE_SLICES = 4
    # emission schedule: group index -> list of (slice_idx, phase) where
    # phase is "ab" (both), "a", or "b"
    prep_at = {0: [(2, "ab")], 1: [(3, "ab"), (4, "b")],
               2: [(5, "a")], 3: [(5, "b")], 4: [(6, "a")],
               6: [(6, "b")]}

    with tile.TileContext(nc) as tc:
        with (
            tc.tile_pool(name="const", bufs=1) as const,
            tc.tile_pool(name="pre", bufs=1) as pre,
            tc.tile_pool(name="lhs", bufs=4) as lhsp,
            tc.tile_pool(name="outs", bufs=3) as outp,
            tc.tile_pool(name="ps", bufs=2, space="PSUM") as psp,
        ):
            xr = pre.tile([P, NB, TB], f32)
            sf = pre.tile([P, NB, TB], f32)
            neg = pre.tile([P, NB, TB], f32)
            sn = pre.tile([P, NB, TB], f32)
            u1 = pre.tile([P, NB, TB], f32)
            u2 = pre.tile([P, NB, TB], f32)
            sg = pre.tile([P, NB, TB], f32)
            x2 = pre.tile([P, NB, TB], f32)
            ve = pre.tile([P, NB, TB], f32)
            rc = pre.tile([P, NB, TB], f32)
            rq = pre.tile([P, NB, TB], f32)
            tt = pre.tile([P, NB, TB], f32)
            v1 = pre.tile([P, NB, TB], f32)
            v2 = pre.tile([P, NB, TB], f32)
            v3 = pre.tile([P, NB, TB], f32)
            q0 = pre.tile([P, NB, TB], f32)
            m4 = pre.tile([P, NB, TB, KD], f16)
            wv = pre.tile([P, 8], f32)
            wa = pre.tile([P, 8], f32)

            # input DMAs: first prep slice's columns first
            nc.sync.dma_start(xr[:, 0:1, :], x_d[:, 0:1, :])
            nc.scalar.dma_start(sf[:, 0:1, :], s_d[:, 0:1, :])
            nc.sync.dma_start(xr[:, 1:NB, :], x_d[:, 1:NB, :])
            nc.scalar.dma_start(sf[:, 1:NB, :], s_d[:, 1:NB, :])
            cm = const.tile([P, TD], f16)
            nc.scalar.dma_start(cm[:], c_d[:])

            # engine warmups during input-DMA latency: wake DVE/GpSimd, and
            # load the ACT Copy+Rsqrt tables before first real use
            nc.vector.memset(wv[:], 1.0)
            nc.vector.tensor_scalar(wv[:], wv[:], 1.0, None, mul)
            nc.vector.tensor_tensor(wv[:], wv[:], wv[:], mul)
            nc.gpsimd.tensor_scalar(wa[:], wv[:], 1.0, None, mul)
            nc.scalar.copy(wa[:], wv[:])
            nc.scalar.activation(wa[:], wv[:],
                                 mybir.ActivationFunctionType.Sqrt)

            # m4 pad planes (k in [KU,KD)) are left uninitialized: only the
            # NaN-safe StreamTranspose reads them, and the K=5 matmuls
            # never touch lh rows 5-31.

            drain_ctr = [0]
            DRAIN_PAT = (0, 1, 0, 0, 1)   # 0 = ACT, 1 = DVE
            N_RAMP_ACT = 2                # first drains forced to ACT

            def emit_prep_a(si):
                c0, gsz = slices[si]
                cs = slice(c0 // TB, (c0 + gsz) // TB)
                e = nc.vector if si < N_DVE_SLICES else nc.gpsimd
                # neg = x<0 ; sig2 = (a + b*neg) + st*(c + d*neg)
                e.tensor_scalar(neg[:, cs, :], xr[:, cs, :], 0.0,
                                None, mybir.AluOpType.is_lt)
                e.tensor_scalar(u1[:, cs, :], neg[:, cs, :], b, a,
                                mul, add)
                e.tensor_scalar(u2[:, cs, :], neg[:, cs, :], d, c,
                                mul, add)
                e.tensor_tensor(sn[:, cs, :], sf[:, cs, :],
                                u2[:, cs, :], mul)
                e.tensor_tensor(sg[:, cs, :], u1[:, cs, :],
                                sn[:, cs, :], add)
                e.tensor_tensor(x2[:, cs, :], xr[:, cs, :],
                                xr[:, cs, :], mul)
                e.tensor_tensor(ve[:, cs, :], x2[:, cs, :],
                                sg[:, cs, :], mul)
                e.tensor_scalar(ve[:, cs, :], ve[:, cs, :], EPS,
                                None, add)

            def emit_prep_b(si):
                c0, gsz = slices[si]
                cs = slice(c0 // TB, (c0 + gsz) // TB)
                e = nc.vector if si < N_DVE_SLICES else nc.gpsimd
                # t = sqrt(x^2 / (x^2*sig2 + eps))
                nc.vector.reciprocal(rc[:, cs, :], ve[:, cs, :])
                e.tensor_tensor(rq[:, cs, :], x2[:, cs, :], rc[:, cs, :],
                                mul)
                nc.scalar.activation(tt[:, cs, :], rq[:, cs, :],
                                     mybir.ActivationFunctionType.Sqrt)
                # one-hot planes: p0 = t(1-s)(1-n), p1 = t(1-s)n,
                # p2 = ts(1-n), p3 = tsn, p4 = 1
                e.tensor_tensor(v1[:, cs, :], tt[:, cs, :],
                                neg[:, cs, :], mul)
                e.tensor_tensor(v2[:, cs, :], tt[:, cs, :],
                                sf[:, cs, :], mul)
                e.tensor_tensor(v3[:, cs, :], v2[:, cs, :],
                                neg[:, cs, :], mul)
                e.tensor_copy(m4[:, cs, :, 3], v3[:, cs, :])
                e.tensor_tensor(m4[:, cs, :, 1], v1[:, cs, :],
                                v3[:, cs, :], sub)
                e.tensor_tensor(m4[:, cs, :, 2], v2[:, cs, :],
                                v3[:, cs, :], sub)
                e.tensor_tensor(q0[:, cs, :], tt[:, cs, :],
                                v1[:, cs, :], sub)
                e.tensor_tensor(q0[:, cs, :], q0[:, cs, :],
                                v2[:, cs, :], sub)
                e.tensor_tensor(m4[:, cs, :, 0], q0[:, cs, :],
                                v3[:, cs, :], add)
                e.memset(m4[:, cs, :, 4], 1.0)

            def emit_prep(si, phase="ab"):
                if "a" in phase:
                    emit_prep_a(si)
                if "b" in phase:
                    emit_prep_b(si)

            def emit_group(h):
                c0, gsz = groups[h]
                outt = outp.tile([P, G, TD], f16, tag="outt")
                for t0 in range(0, gsz, TB):
                    t = (c0 + t0) // TB
                    lh = lhsp.tile([P, P], f16, tag="lh")
                    nc.vector.transpose(lh[:], m4[:, t, :, :])
                    pp = psp.tile([P, TB, TD], f32, tag="pp")
                    for j in range(TB):
                        nc.tensor.matmul(
                            pp[:, j, :],
                            lh[KD * j:KD * j + KU, :],
                            cm[KD * j:KD * j + KU, :],
                            start=True, stop=True,
                            tile_position=(KD * j, 0))
                    dst = outt[:, t0:t0 + TB, :]
                    ctr = drain_ctr[0]
                    use_dve = (ctr >= N_RAMP_ACT
                               and DRAIN_PAT[ctr % len(DRAIN_PAT)])
                    if use_dve:
                        nc.vector.tensor_copy(dst, pp[:])
                    else:
                        nc.scalar.copy(dst, pp[:])
                    drain_ctr[0] += 1
                    if t0 == 0:
                        for si, phase in prep_at.get(h, ()):
                            emit_prep(si, phase)
                nc.sync.dma_start(y_d[:, c0:c0 + gsz, :], outt[:, 0:gsz, :])

            emit_prep(0)
            emit_prep(1)
            emit_prep(4, "a")
            for h in range(len(groups)):
                emit_group(h)
    nc.compile()
    return nc


def _make_in_maps(inputs, cmat):
    x = np.ascontiguousarray(np.asarray(inputs["x"], dtype=np.float32)).reshape(B)
    st = np.asarray(inputs["shape_type"]).astype(np.float32).reshape(B)
    in_maps = []
    for i in range(N_CORES):
        sl = slice(i * RPC, (i + 1) * RPC)
        # [q = 32j+a, t, rb] <- row (32rb + a)*CPB + 4t + j
        xq = x[sl].reshape(TB, 32, NB, TB).transpose(3, 1, 2, 0).reshape(
            P, NB, TB)
        sq = st[sl].reshape(TB, 32, NB, TB).transpose(3, 1, 2, 0).reshape(
            P, NB, TB)
        in_maps.append({
            "x": np.ascontiguousarray(xq),
            "st": np.ascontiguousarray(sq),
            "cmat": cmat,
        })
    return in_maps


def _get_nc(sig2):
    key = tuple(np.round(sig2, 12))
    if key not in _CACHE:
        _CACHE[key] = _build_nc(sig2)
    return _CACHE[key]


def _get_runner(nc):
    """Cached jit-compiled SPMD executor for `nc` (same mechanics as
    concourse.bass2jax.run_bass_via_pjrt, memoized so repeated kernel()
    calls skip jax re-tracing)."""
    if hasattr(nc, "_cached_runner"):
        return nc._cached_runner
    import jax
    from jax.experimental.shard_map import shard_map
    from jax.sharding import Mesh, PartitionSpec

    import concourse.mybir as mybir
    from concourse import bass2jax

    bass2jax.install_neuronx_cc_hook()

    part_name = (nc.partition_id_tensor.name
                 if nc.partition_id_tensor else None)
    in_names, out_names, out_avals = [], [], []
    for alloc in nc.m.functions[0].allocations:
        if not isinstance(alloc, mybir.MemoryLocationSet):
            continue
        name = alloc.memorylocations[0].name
        if alloc.kind == "ExternalInput":
            if name != part_name:
                in_names.append(name)
        elif alloc.kind == "ExternalOutput":
            out_names.append(name)
            out_avals.append(jax.core.ShapedArray(
                tuple(alloc.tensor_shape), mybir.dt.np(alloc.dtype)))
    n_params = len(in_names)
    all_names = in_names + out_names
    if part_name is not None:
        all_names = all_names + [part_name]
    donate = tuple(range(n_params, n_params + len(out_names)))

    def _body(*args):
        operands = list(args)
        if part_name is not None:
            operands.append(bass2jax.partition_id_tensor())
        return tuple(bass2jax._bass_exec_p.bind(
            *operands,
            out_avals=tuple(out_avals),
            in_names=tuple(all_names),
            out_names=tuple(out_names),
            lowering_input_output_aliases=(),
            sim_require_finite=True,
            sim_require_nnan=True,
            nc=nc,
        ))

    devices = jax.devices()[:N_CORES]
    mesh = Mesh(np.asarray(devices), ("core",))
    sharded = jax.jit(
        shard_map(_body, mesh=mesh,
                  in_specs=(PartitionSpec("core"),) * (n_params + len(out_names)),
                  out_specs=(PartitionSpec("core"),) * len(out_names),
                  check_rep=False),
        donate_argnums=donate, keep_unused=True)
    runner = (sharded, in_names, out_names, out_avals)
    nc._cached_runner = runner
    return runner


def _run_spmd(nc, in_maps):
    sharded, in_names, out_names, out_avals = _get_runner(nc)
    concat_in = [
        np.concatenate([np.asarray(m[name])[None] for m in in_maps], axis=0)
        .reshape(N_CORES * in_maps[0][name].shape[0],
                 *in_maps[0][name].shape[1:])
        for name in in_names
    ]
    concat_zeros = [
        np.zeros((N_CORES * a.shape[0], *a.shape[1:]), a.dtype)
        for a in out_avals
    ]
    out_arrs = sharded(*concat_in, *concat_zeros)
    return {
        name: np.asarray(out_arrs[i]).reshape(
            N_CORES, *out_avals[i].shape)
        for i, name in enumerate(out_names)
    }


def kernel(**inputs) -> np.ndarray:
    if not _assumptions_hold(inputs):
        return _fallback_numpy(inputs)

    cmat, sig2 = _towers_collapse(inputs)
    nc = _get_nc(sig2)
    in_maps = _make_in_maps(inputs, cmat)
    y = _run_spmd(nc, in_maps)["y"]            # [N_CORES, P, CPB, TD] f16
    return np.ascontiguousarray(y.reshape(B, TD)).astype(np.float32)


# revision 37
# speedup vs baseline: 1.0250x; 1.0012x over previous
"""Trainium2 Bass kernel for nn_FCLModule_74131135529089 (moe_routing).

Module structure (B=262144 rows, input dim 1):
    circle/rect expert towers 1->32->64->256 (relu, zero biases)
    per-row select by shape_type, stage2 256->256 relu + residual,
    stage3 256->512 relu, 512->512, LayerNorm(512).

All bias vectors in this module are zero and every stage before the
LayerNorm is positively homogeneous in x: for each row,
    h2(x) = |x| * H[k],   k = 2*shape_type + (x < 0),
where H[k] in R^512 is the full pre-LayerNorm output of the network at
x = +-1 for each expert.  The LayerNorm collapses to
    out = C[k] * t + ln_b,  C[k] = (H[k]-mean(H[k]))*ln_g,
    t = |x| / sqrt(x^2 * var(H[k]) + eps).

Device kernel: per 128-row chunk build a [5, 128] fp16 stationary
(4 one-hot*t rows + ones row) and multiply with the constant [5, 512]
fp16 matrix (C rows + ln_b) on the tensor engine; accumulate fp32 in
PSUM, drain to SBUF as fp16 (ACT/DVE alternating), DMA fp16 to HBM and
upcast to fp32 on the host.  fp16 output keeps rel err ~1e-3, well
under the 2e-2 gate, and halves HBM write traffic (the roofline).

The stationaries are produced without tensor-engine transposes: inputs
are host-permuted so that the per-row coefficient planes, written by
the vector engine as [128, 4, 32]-blocked tiles, become per-chunk
stationaries under the DVE 32x32 StreamTranspose (one per 4 chunks).

Sharding: pure data parallel over the batch dim, 8 cores x 32768 rows.
If structural assumptions are violated (nonzero biases / shape_type
outside {0,1}) we fall back to dense numpy evaluation.
"""

import numpy as np

B = 262144
TD = 512
N_CORES = 8
RPC = B // N_CORES          # rows per core = 32768
P = 128                     # SBUF partitions
CPB = RPC // P              # chunks per core = 256 (row r = p*CPB + c)
TB = 4                      # chunks per transpose batch
NB = CPB // TB              # transpose batches = 64
KD = 32                     # stream-transpose block / stationary slot
KU = 5                      # used stationary rows (4 masks*t + ones)
G = 32                      # chunks per output DMA group
EPS = 1e-5

_CACHE: dict = {}


def _towers_collapse(inputs):
    """Host-side constant folding (float64): returns the replicated fp16
    constant matrix [128, TD] (rows 32j+k: k<4 -> C[k], k=4 -> ln_b) and
    sig2 [4] f64, for k = 2*shape_type + (x<0) in order
    (c,+),(c,-),(r,+),(r,-)."""
    W = {k: np.asarray(v, dtype=np.float64) for k, v in inputs.items()}
    H = []
    for e in ("c", "r"):
        for sign in (1.0, -1.0):
            v = np.array([[sign]])
            for li in ("1", "2", "3"):
                v = np.maximum(v @ W[e + "w" + li] + W[e + "b" + li], 0.0)
            x2 = np.maximum(v @ W["s2w"] + W["s2b"], 0.0) + v
            h = np.maximum(x2 @ W["w3a"] + W["b3a"], 0.0)
            H.append((h @ W["w3b"] + W["b3b"])[0])
    H = np.stack(H)                                   # [4, TD]
    mu = H.mean(axis=1, keepdims=True)
    sig2 = H.var(axis=1)                              # [4]
    C = (H - mu) * W["ln_g"][None, :]                 # [4, TD]
    # one-hot basis: plane k = t*mask_k keeps the fp16 rounding error
    # relative to the single active C row (affine differences would not)
    blk = np.zeros((KD, TD), np.float16)
    blk[0:4] = C.astype(np.float16)
    blk[4] = W["ln_b"].astype(np.float16)
    cmat = np.tile(blk, (TB, 1))                      # [128, TD] f16
    return np.ascontiguousarray(cmat), sig2


def _assumptions_hold(inputs):
    for name in ("cb1", "cb2", "cb3", "rb1", "rb2", "rb3", "s2b", "b3a", "b3b"):
        if np.any(np.asarray(inputs[name]) != 0):
            return False
    st = np.asarray(inputs["shape_type"])
    if not np.isin(st, (0, 1)).all():
        return False
    x = np.asarray(inputs["x"])
    return bool(np.isfinite(x).all()) and x.shape == (B, 1) and st.shape == (B, 1)


def _fallback_numpy(inputs):
    f = {k: np.asarray(v, dtype=np.float32) for k, v in inputs.items()}

    def tower(h, w1, b1, w2, b2, w3, b3):
        h = np.maximum(h @ w1 + b1, 0)
        h = np.maximum(h @ w2 + b2, 0)
        return np.maximum(h @ w3 + b3, 0)

    x = f["x"]
    circle = tower(x, f["cw1"], f["cb1"], f["cw2"], f["cb2"], f["cw3"], f["cb3"])
    rect = tower(x, f["rw1"], f["rb1"], f["rw2"], f["rb2"], f["rw3"], f["rb3"])
    mask = np.asarray(inputs["shape_type"]) < 0.5
    x1 = np.where(mask, circle, rect)
    x2 = np.maximum(x1 @ f["s2w"] + f["s2b"], 0) + x1
    h = np.maximum(x2 @ f["w3a"] + f["b3a"], 0)
    h = h @ f["w3b"] + f["b3b"]
    mu = h.mean(axis=-1, keepdims=True)
    var = h.var(axis=-1, keepdims=True)
    return ((h - mu) / np.sqrt(var + EPS) * f["ln_g"] + f["ln_b"]).astype(np.float32)


def _build_nc(sig2):
    import concourse.bacc as bacc
    import concourse.bass as bass
    import concourse.mybir as mybir
    import concourse.tile as tile

    f32 = mybir.dt.float32
    f16 = mybir.dt.float16
    a = float(sig2[0])
    b = float(sig2[1] - sig2[0])
    c = float(sig2[2] - sig2[0])
    d = float(sig2[3] - sig2[2] - sig2[1] + sig2[0])
    mul = mybir.AluOpType.mult
    add = mybir.AluOpType.add
    sub = mybir.AluOpType.subtract

    nc = bacc.Bacc("TRN2", target_bir_lowering=False, debug=False,
                   num_devices=N_CORES)
    # host-permuted inputs: element [q, t, rb] = row (32*rb + q%32)*CPB
    # + 4*t + q//32  (q = 32j + a selects chunk-within-batch j, row-mod a)
    x_d = nc.dram_tensor("x", [P, NB, TB], f32, kind="ExternalInput").ap()
    s_d = nc.dram_tensor("st", [P, NB, TB], f32, kind="ExternalInput").ap()
    c_d = nc.dram_tensor("cmat", [P, TD], f16, kind="ExternalInput").ap()
    y_d = nc.dram_tensor("y", [P, CPB, TD], f16, kind="ExternalOutput").ap()

    # output DMA groups
    groups = [(0, 4), (4, 4), (8, 8), (16, 16), (32, 32)]
    g0 = 64
    while g0 < CPB:
        groups.append((g0, G))
        g0 += G
    # prep slices (chunk ranges).  Small head slices run fully on DVE for
    # latency; big tail slices run on GpSimd (idle engine) so DVE keeps
    # only transposes + PSUM drains, immune to the shared-SBUF-port
    # contention GpSimd inflicts on DVE SBUF-read ops.  GpSimd slices are
    # split: phase A (chain up to ve) emitted EARLY, phase B (sqrt +
    # planes) emitted LATE so the in-order ACT queue never head-of-line
    # blocks drains behind a sqrt whose input chain is still running.
    slices = [(0, 4), (4, 4), (8, 8), (16, 16),
              (32, 64), (96, 64), (160, 96)]
    N_DVE_SLICES = 4
    # emission schedule: group index -> list of (slice_idx, phase) where
    # phase is "ab" (both), "a", or "b"
    prep_at = {0: [(2, "ab")], 1: [(3, "ab"), (4, "b")],
               2: [(5, "a")], 3: [(5, "b")], 4: [(6, "a")],
               6: [(6, "b")]}

    with tile.TileContext(nc) as tc:
        with (
            tc.tile_pool(name="const", bufs=1) as const,
            tc.tile_pool(name="pre", bufs=1) as pre,
            tc.tile_pool(name="lhs", bufs=6) as lhsp,
            tc.tile_pool(name="outs", bufs=3) as outp,
            tc.tile_pool(name="ps", bufs=2, space="PSUM") as psp,
        ):
            xr = pre.tile([P, NB, TB], f32)
            sf = pre.tile([P, NB, TB], f32)
            neg = pre.tile([P, NB, TB], f32)
            sn = pre.tile([P, NB, TB], f32)
            u1 = pre.tile([P, NB, TB], f32)
            u2 = pre.tile([P, NB, TB], f32)
            sg = pre.tile([P, NB, TB], f32)
            x2 = pre.tile([P, NB, TB], f32)
            ve = pre.tile([P, NB, TB], f32)
            rc = pre.tile([P, NB, TB], f32)
            rq = pre.tile([P, NB, TB], f32)
            tt = pre.tile([P, NB, TB], f32)
            v1 = pre.tile([P, NB, TB], f32)
            v2 = pre.tile([P, NB, TB], f32)
            v3 = pre.tile([P, NB, TB], f32)
            q0 = pre.tile([P, NB, TB], f32)
            m4 = pre.tile([P, NB, TB, KD], f16)
            wv = pre.tile([P, 8], f32)
            wa = pre.tile([P, 8], f32)

            # input DMAs: first prep slice's columns first
            nc.sync.dma_start(xr[:, 0:1, :], x_d[:, 0:1, :])
            nc.scalar.dma_start(sf[:, 0:1, :], s_d[:, 0:1, :])
            nc.sync.dma_start(xr[:, 1:NB, :], x_d[:, 1:NB, :])
            nc.scalar.dma_start(sf[:, 1:NB, :], s_d[:, 1:NB, :])
            cm = const.tile([P, TD], f16)
            nc.scalar.dma_start(cm[:], c_d[:])

            # engine warmups during input-DMA latency: wake DVE/GpSimd, and
            # load the ACT Copy+Rsqrt tables before first real use
            nc.vector.memset(wv[:], 1.0)
            nc.vector.tensor_scalar(wv[:], wv[:], 1.0, None, mul)
            nc.vector.tensor_tensor(wv[:], wv[:], wv[:], mul)
            nc.gpsimd.tensor_scalar(wa[:], wv[:], 1.0, None, mul)
            nc.scalar.copy(wa[:], wv[:])
            nc.scalar.activation(wa[:], wv[:],
                                 mybir.ActivationFunctionType.Sqrt)

            # m4 pad planes (k in [KU,KD)) are left uninitialized: only the
            # NaN-safe StreamTranspose reads them, and the K=5 matmuls
            # never touch lh rows 5-31.

            drain_ctr = [0]
            DRAIN_PAT = (0, 1, 0, 0, 1)   # 0 = ACT, 1 = DVE
            N_RAMP_ACT = 2                # first drains forced to ACT

            def emit_prep_a(si):
                c0, gsz = slices[si]
                cs = slice(c0 // TB, (c0 + gsz) // TB)
                e = nc.vector if si < N_DVE_SLICES else nc.gpsimd
                # neg = x<0 ; sig2 = (a + b*neg) + st*(c + d*neg)
                e.tensor_scalar(neg[:, cs, :], xr[:, cs, :], 0.0,
                                None, mybir.AluOpType.is_lt)
                e.tensor_scalar(u1[:, cs, :], neg[:, cs, :], b, a,
                                mul, add)
                e.tensor_scalar(u2[:, cs, :], neg[:, cs, :], d, c,
                                mul, add)
                e.tensor_tensor(sn[:, cs, :], sf[:, cs, :],
                                u2[:, cs, :], mul)
                e.tensor_tensor(sg[:, cs, :], u1[:, cs, :],
                                sn[:, cs, :], add)
                e.tensor_tensor(x2[:, cs, :], xr[:, cs, :],
                                xr[:, cs, :], mul)
                e.tensor_tensor(ve[:, cs, :], x2[:, cs, :],
                                sg[:, cs, :], mul)
                e.tensor_scalar(ve[:, cs, :], ve[:, cs, :], EPS,
                                None, add)

            def emit_prep_b(si):
                c0, gsz = slices[si]
                cs = slice(c0 // TB, (c0 + gsz) // TB)
                e = nc.vector if si < N_DVE_SLICES else nc.gpsimd
                # t = sqrt(x^2 / (x^2*sig2 + eps))
                nc.vector.reciprocal(rc[:, cs, :], ve[:, cs, :])
                e.tensor_tensor(rq[:, cs, :], x2[:, cs, :], rc[:, cs, :],
                                mul)
                nc.scalar.activation(tt[:, cs, :], rq[:, cs, :],
                                     mybir.ActivationFunctionType.Sqrt)
                # one-hot planes: p0 = t(1-s)(1-n), p1 = t(1-s)n,
                # p2 = ts(1-n), p3 = tsn, p4 = 1
                e.tensor_tensor(v1[:, cs, :], tt[:, cs, :],
                                neg[:, cs, :], mul)
                e.tensor_tensor(v2[:, cs, :], tt[:, cs, :],
                                sf[:, cs, :], mul)
                e.tensor_tensor(v3[:, cs, :], v2[:, cs, :],
                                neg[:, cs, :], mul)
                e.tensor_copy(m4[:, cs, :, 3], v3[:, cs, :])
                e.tensor_tensor(m4[:, cs, :, 1], v1[:, cs, :],
                                v3[:, cs, :], sub)
                e.tensor_tensor(m4[:, cs, :, 2], v2[:, cs, :],
                                v3[:, cs, :], sub)
                e.tensor_tensor(q0[:, cs, :], tt[:, cs, :],
                                v1[:, cs, :], sub)
                e.tensor_tensor(q0[:, cs, :], q0[:, cs, :],
                                v2[:, cs, :], sub)
                e.tensor_tensor(m4[:, cs, :, 0], q0[:, cs, :],
                                v3[:, cs, :], add)
                e.memset(m4[:, cs, :, 4], 1.0)

            def emit_prep(si, phase="ab"):
                if "a" in phase:
                    emit_prep_a(si)
                if "b" in phase:
                    emit_prep_b(si)

            def emit_group(h):
                c0, gsz = groups[h]
                outt = outp.tile([P, G, TD], f16, tag="outt")
                for t0 in range(0, gsz, TB):
                    t = (c0 + t0) // TB
                    lh = lhsp.tile([P, P], f16, tag="lh")
                    nc.vector.transpose(lh[:], m4[:, t, :, :])
                    pp = psp.tile([P, TB, TD], f32, tag="pp")
                    for j in range(TB):
                        nc.tensor.matmul(
                            pp[:, j, :],
                            lh[KD * j:KD * j + KU, :],
                            cm[KD * j:KD * j + KU, :],
                            start=True, stop=True,
                            tile_position=(KD * j, 0))
                    dst = outt[:, t0:t0 + TB, :]
                    ctr = drain_ctr[0]
                    use_dve = (ctr >= N_RAMP_ACT
                               and DRAIN_PAT[ctr % len(DRAIN_PAT)])
                    if use_dve:
                        nc.vector.tensor_copy(dst, pp[:])
                    else:
                        nc.scalar.copy(dst, pp[:])
                    drain_ctr[0] += 1
                    if t0 == 0:
                        for si, phase in prep_at.get(h, ()):
                            emit_prep(si, phase)
                nc.sync.dma_start(y_d[:, c0:c0 + gsz, :], outt[:, 0:gsz, :])

            emit_prep(0)
            emit_prep(1)
            emit_prep(4, "a")
            for h in range(len(groups)):
                emit_group(h)
    nc.compile()
    return nc


def _make_in_maps(inputs, cmat):
    x = np.ascontiguousarray(np.asarray(inputs["x"], dtype=np.float32)).reshape(B)
    st = np.asarray(inputs["shape_type"]).astype(np.float32).reshape(B)
    in_maps = []
    for i in range(N_CORES):
        sl = slice(i * RPC, (i + 1) * RPC)
        # [q = 32j+a, t, rb] <- row (32rb + a)*CPB + 4t + j
        xq = x[sl].reshape(TB, 32, NB, TB).transpose(3, 1, 2, 0).reshape(
            P, NB, TB)
        sq = st[sl].reshape(TB, 32, NB, TB).transpose(3, 1, 2, 0).reshape(
            P, NB, TB)
        in_maps.append({
            "x": np.ascontiguousarray(xq),
            "st": np.ascontiguousarray(sq),
            "cmat": cmat,
        })
    return in_maps


def _get_nc(sig2):
    key = tuple(np.round(sig2, 12))
    if key not in _CACHE:
        _CACHE[key] = _build_nc(sig2)
    return _CACHE[key]


def _get_runner(nc):
    """Cached jit-compiled SPMD executor for `nc` (same mechanics as
    concourse.bass2jax.run_bass_via_pjrt, memoized so repeated kernel()
    calls skip jax re-tracing)."""
    if hasattr(nc, "_cached_runner"):
        return nc._cached_runner
    import jax
    from jax.experimental.shard_map import shard_map
    from jax.sharding import Mesh, PartitionSpec

    import concourse.mybir as mybir
    from concourse import bass2jax

    bass2jax.install_neuronx_cc_hook()

    part_name = (nc.partition_id_tensor.name
                 if nc.partition_id_tensor else None)
    in_names, out_names, out_avals = [], [], []
    for alloc in nc.m.functions[0].allocations:
        if not isinstance(alloc, mybir.MemoryLocationSet):
            continue
        name = alloc.memorylocations[0].name
        if alloc.kind == "ExternalInput":
            if name != part_name:
                in_names.append(name)
        elif alloc.kind == "ExternalOutput":
            out_names.append(name)
            out_avals.append(jax.core.ShapedArray(
                tuple(alloc.tensor_shape), mybir.dt.np(alloc.dtype)))
    n_params = len(in_names)
    all_names = in_names + out_names
    if part_name is not None:
        all_names = all_names + [part_name]
    donate = tuple(range(n_params, n_params + len(out_names)))

    def _body(*args):
        operands = list(args)
        if part_name is not None:
            operands.append(bass2jax.partition_id_tensor())
        return tuple(bass2jax._bass_exec_p.bind(
            *operands,
            out_avals=tuple(out_avals),
            in_names=tuple(all_names),
            out_names=tuple(out_names),
            lowering_input_output_aliases=(),
            sim_require_finite=True,
            sim_require_nnan=True,
            nc=nc,
        ))

    devices = jax.devices()[:N_CORES]
    mesh = Mesh(np.asarray(devices), ("core",))
    sharded = jax.jit(
        shard_map(_body, mesh=mesh,
                  in_specs=(PartitionSpec("core"),) * (n_params + len(out_names)),
                  out_specs=(PartitionSpec("core"),) * len(out_names),
                  check_rep=False),
        donate_argnums=donate, keep_unused=True)
    runner = (sharded, in_names, out_names, out_avals)
    nc._cached_runner = runner
    return runner


def _run_spmd(nc, in_maps):
    sharded, in_names, out_names, out_avals = _get_runner(nc)
    concat_in = [
        np.concatenate([np.asarray(m[name])[None] for m in in_maps], axis=0)
        .reshape(N_CORES * in_maps[0][name].shape[0],
                 *in_maps[0][name].shape[1:])
        for name in in_names
    ]
    concat_zeros = [
        np.zeros((N_CORES * a.shape[0], *a.shape[1:]), a.dtype)
        for a in out_avals
    ]
    out_arrs = sharded(*concat_in, *concat_zeros)
    return {
        name: np.asarray(out_arrs[i]).reshape(
            N_CORES, *out_avals[i].shape)
        for i, name in enumerate(out_names)
    }


def kernel(**inputs) -> np.ndarray:
    if not _assumptions_hold(inputs):
        return _fallback_numpy(inputs)

    cmat, sig2 = _towers_collapse(inputs)
    nc = _get_nc(sig2)
    in_maps = _make_in_maps(inputs, cmat)
    y = _run_spmd(nc, in_maps)["y"]            # [N_CORES, P, CPB, TD] f16
    return np.ascontiguousarray(y.reshape(B, TD)).astype(np.float32)
